# revision 19
# baseline (speedup 1.0000x reference)
"""GroupedQueryAttention Trainium2 Bass kernel (8 NeuronCores, SPMD), v2.

Same collapsed-GQA math as v1 (see kernel.py docstring): the reference's
tiled K/V + full-2048-dim attention reduces to one 512-dim attention with
summed WQ row-blocks and summed proj row-blocks.

v2 changes vs v1 (290us -> ~249us):
- fp16 matmul operands everywhere (x, weights, Q/K/V, E, Z, proj).
  PSUM accumulation stays fp32; softmax scalars stay fp32.  Host-side
  numerics check: rel err 3.1e-3 (same as fp32r) << 2e-2 tolerance.
  Halves DMA traffic and SBUF footprint; E^T/Z^T PE transposes run at
  1 cycle/row instead of 2.
- Output projection interleaved into the attention software pipeline
  (o(g-1) emitted after z(g)) to remove the attention->projection
  transition bubble; 2-block software-pipeline lag keeps PE fed.
- Chunk-granular causality: even row blocks compute their last key tile
  at width 256 (the upper 256 keys are fully masked anyway).
- K PSUM drains on scalar (bias add) run in parallel with V drains on
  vector; output DMA streamed per 512-column chunk.
- First compute-feeding DMAs (wk0/wv0/xt0) issued before misc constant
  loads to shrink the kernel lead-in.
(DMA XBAR transposes and remote_dma pair-exchange of K/V were tried and
rejected: the former is racy/wrong-output-prone here, the latter's
SWDGE desc-gen ucode faults on this runtime.)

Sharding unchanged: 8 cores = 4 batches x 2 interleaved 128-row blocks;
causal key tiles per block [1,1,2,2,3,3,4,4] identical on both pair
cores -> single SPMD program.
"""

import numpy as np

import concourse.bacc as bacc
import concourse.mybir as mybir
from concourse.tile import TileContext
from concourse.bass_utils import run_bass_kernel_spmd

B, T, D = 4, 2048, 2048
HD = 512                 # collapsed head dim
NCORES = 8
RB = 8                   # 128-row blocks per core
KT_TILES = [1, 1, 2, 2, 3, 3, 4, 4]   # causal 512-wide key tiles per block
DCH = D // 128           # 16 contraction chunks
dt = mybir.dt
NEG = -30000.0           # fp16-safe mask value


def build_kernel():
    nc = bacc.Bacc(None, target_bir_lowering=False)

    xT_d = nc.dram_tensor("xT", [D, T], dt.float16, kind="ExternalInput")
    xTq_d = nc.dram_tensor("xTq", [D, 1024], dt.float16, kind="ExternalInput")
    WKT_d = nc.dram_tensor("WKT", [D, HD], dt.float16, kind="ExternalInput")
    WVT_d = nc.dram_tensor("WVT", [D, HD], dt.float16, kind="ExternalInput")
    WQT_d = nc.dram_tensor("WQT", [D, HD], dt.float16, kind="ExternalInput")
    PRJ_d = nc.dram_tensor("PRJ", [HD, D], dt.float16, kind="ExternalInput")
    bK_d = nc.dram_tensor("bK", [HD, 1], dt.float32, kind="ExternalInput")
    bQ_d = nc.dram_tensor("bQ", [HD, 1], dt.float32, kind="ExternalInput")
    BVF_d = nc.dram_tensor("BVF", [128, HD], dt.float16, kind="ExternalInput")
    IDT_d = nc.dram_tensor("IDT", [128, 128], dt.float16, kind="ExternalInput")
    MSK_d = nc.dram_tensor("MSK", [RB, 128, 512], dt.float32, kind="ExternalInput")
    OUT_d = nc.dram_tensor("out", [1024, D], dt.float32, kind="ExternalOutput")

    Act = mybir.ActivationFunctionType
    Ax = mybir.AxisListType

    with TileContext(nc) as tc:
        with tc.tile_pool(name="persist", bufs=1) as pp:
            # ---- persistent tiles ------------------------------------------
            KT_sb = [pp.tile([128, T], dt.float16, tag=f"KT{h}", name=f"KT{h}")
                     for h in range(4)]
            V_sb = [pp.tile([128, HD], dt.float16, tag=f"V{k}", name=f"V{k}")
                    for k in range(16)]
            QT_sb = [pp.tile([128, 1024], dt.float16, tag=f"QT{h}", name=f"QT{h}")
                     for h in range(4)]
            # Z^T per row block, chunk-transposed: hd-chunk h at columns
            # 128*h. One contiguous [128,512] tile per block — XBAR DMA
            # transpose needs a contiguous destination (strided slices of a
            # wider tile produce wrong output on hardware).
            ZT_blk = [pp.tile([128, 512], dt.float16, tag=f"ZTb{g}",
                              name=f"ZTb{g}") for g in range(RB)]
            bvf = pp.tile([128, HD], dt.float16, tag="bvf")
            ident = pp.tile([128, 128], dt.float16, tag="ident")

            # ---- stage P: projections --------------------------------------
            # x^T streamed by 128-D chunk; weights streamed with the first key
            # group then resident. Per 512-key group: 4 PSUM banks accumulate
            # K^T hd-tiles ([hd, keys], bias per partition) and 4 banks
            # accumulate V key-blocks directly in [keys, hd] layout (xt chunk
            # as lhsT); V bias enters as a rank-1 ones x bV matmul.
            with tc.tile_pool(name="wpool", bufs=1) as wp, \
                 tc.tile_pool(name="xstream", bufs=6) as xp, \
                 tc.tile_pool(name="psP", bufs=1, space="PSUM") as psp:
                wk = [None] * DCH
                wv = [None] * DCH
                for rt in range(4):                      # key 512-col groups
                    kps = [psp.tile([128, 512], dt.float32, tag=f"kps{h}",
                                    name=f"kps{h}") for h in range(4)]
                    vps = [psp.tile([128, 512], dt.float32, tag=f"vps{j}",
                                    name=f"vps{j}") for j in range(4)]
                    for c in range(DCH):
                        xt = xp.tile([128, 512], dt.float16, tag="xs", name="xt")
                        xeng = nc.scalar if c % 2 == 0 else nc.sync
                        xeng.dma_start(
                            out=xt[:],
                            in_=xT_d[128 * c:128 * c + 128, 512 * rt:512 * rt + 512])
                        if rt == 0:   # stream weights in with the first pass
                            tk = wp.tile([128, HD], dt.float16, tag=f"wk{c}",
                                         name=f"wk{c}")
                            nc.sync.dma_start(
                                out=tk[:], in_=WKT_d[128 * c:128 * c + 128, :])
                            wk[c] = tk
                            tv = wp.tile([128, HD], dt.float16, tag=f"wv{c}",
                                         name=f"wv{c}")
                            nc.scalar.dma_start(
                                out=tv[:], in_=WVT_d[128 * c:128 * c + 128, :])
                            wv[c] = tv
                        st = (c == 0)
                        for h in range(4):
                            nc.tensor.matmul(kps[h][:],
                                             wk[c][:, 128 * h:128 * h + 128],
                                             xt[:], start=st, stop=(c == DCH - 1))
                        for j in range(4):
                            nc.tensor.matmul(vps[j][:],
                                             xt[:, 128 * j:128 * j + 128],
                                             wv[c][:], start=st,
                                             stop=(c == DCH - 1))
                    if rt == 0:
                        nc.scalar.dma_start(out=bvf[:], in_=BVF_d[:])
                        nc.scalar.dma_start(out=ident[:], in_=IDT_d[:])
                        bias_sb = {}
                        for nm, src in (("bK", bK_d), ("bQ", bQ_d)):
                            for h in range(4):
                                t = pp.tile([128, 1], dt.float32, tag=f"{nm}{h}",
                                            name=f"{nm}{h}")
                                nc.sync.dma_start(
                                    out=t[:], in_=src[128 * h:128 * h + 128, :])
                                bias_sb[nm, h] = t
                    # drain K on scalar (bias add) and V on vector, in
                    # parallel, so the next group's matmuls get their PSUM
                    # banks back ~2x sooner
                    for h in range(4):
                        nc.scalar.activation(
                            KT_sb[h][:, 512 * rt:512 * rt + 512], kps[h][:],
                            Act.Identity, bias=bias_sb["bK", h][:], scale=1.0)
                    for j in range(4):
                        nc.vector.tensor_add(V_sb[4 * rt + j][:], vps[j][:],
                                             bvf[:])
                # Qc^T for this core's 1024 rows (reuses the wk slots).
                wq = []
                for c in range(DCH):
                    tq = wp.tile([128, HD], dt.float16, tag=f"wk{c}", name=f"wq{c}")
                    nc.scalar.dma_start(out=tq[:], in_=WQT_d[128 * c:128 * c + 128, :])
                    wq.append(tq)
                for rt in range(2):
                    qps = [psp.tile([128, 512], dt.float32, tag=f"kps{h}",
                                    name=f"qps{h}") for h in range(4)]
                    for c in range(DCH):
                        xt = xp.tile([128, 512], dt.float16, tag="xs", name="xt")
                        xeng = nc.sync if c % 2 == 0 else nc.scalar
                        xeng.dma_start(
                            out=xt[:],
                            in_=xTq_d[128 * c:128 * c + 128, 512 * rt:512 * rt + 512])
                        for h in range(4):
                            nc.tensor.matmul(qps[h][:],
                                             wq[c][:, 128 * h:128 * h + 128],
                                             xt[:], start=(c == 0),
                                             stop=(c == DCH - 1))
                    for h in range(4):
                        nc.scalar.activation(
                            QT_sb[h][:, 512 * rt:512 * rt + 512], qps[h][:],
                            Act.Identity, bias=bias_sb["bQ", h][:], scale=1.0)

            # ---- stages A+O: attention with interleaved output proj --------
            with tc.tile_pool(name="attn", bufs=1) as ap, \
                 tc.tile_pool(name="oproj", bufs=1) as op, \
                 tc.tile_pool(name="psA", bufs=1, space="PSUM") as psa:
                # prefetch output-projection weights while attention runs
                prj = []
                for h in range(4):
                    t = op.tile([128, D], dt.float16, tag=f"prj{h}", name=f"prj{h}")
                    nc.sync.dma_start(out=t[:], in_=PRJ_d[128 * h:128 * h + 128, :])
                    prj.append(t)

                state = {}

                def stage_s(g):
                    """S matmuls + mask + per-tile max for row block g.

                    Rows of block g (either pair core) only reach key
                    128*(2g+2)-1, so the last 512-key tile is computed at
                    width 256 for even g (its upper 256 keys are entirely
                    masked) — chunk-granular causality.
                    """
                    ntile = KT_TILES[g]
                    lw = 256 if g % 2 == 0 else 512   # last-tile width
                    mpart = ap.tile([128, 4], dt.float32, tag="mpart", bufs=2,
                                    name="mpart")
                    s_tiles = []
                    for kt in range(ntile):
                        w = lw if kt == ntile - 1 else 512
                        sps = psa.tile([128, 512], dt.float32, tag="sps", bufs=2,
                                       name="sps")
                        for h in range(4):
                            nc.tensor.matmul(
                                sps[:, 0:w], QT_sb[h][:, 128 * g:128 * g + 128],
                                KT_sb[h][:, 512 * kt:512 * kt + w],
                                start=(h == 0), stop=(h == 3))
                        ssb = ap.tile([128, 512], dt.float32, tag="ssb", bufs=9,
                                      name="ssb")
                        if kt == ntile - 1:
                            mt = ap.tile([128, 512], dt.float32, tag="mask",
                                         bufs=2, name="mt")
                            nc.sync.dma_start(out=mt[:, 0:w], in_=MSK_d[g, :, 0:w])
                            nc.vector.tensor_add(ssb[:, 0:w], sps[:, 0:w],
                                                 mt[:, 0:w])
                        elif kt % 2 == 0:
                            nc.scalar.copy(ssb[:, 0:w], sps[:, 0:w])
                        else:
                            nc.vector.tensor_copy(ssb[:, 0:w], sps[:, 0:w])
                        nc.vector.reduce_max(mpart[:, kt:kt + 1], ssb[:, 0:w],
                                             axis=Ax.X)
                        s_tiles.append((ssb, w))
                    state[g] = (s_tiles, mpart)

                def stage_e(g):
                    """negmax + exp + E^T DMA transposes + 1/(sum*sqrt(hs))."""
                    ntile = KT_TILES[g]
                    s_tiles, mpart = state[g]
                    negm = ap.tile([128, 1], dt.float32, tag="negm", bufs=2,
                                   name="negm")
                    nc.vector.reduce_max(negm[:], mpart[:, 0:ntile], axis=Ax.X,
                                         negate=True)
                    esum = ap.tile([128, 4], dt.float32, tag="esum", bufs=2,
                                   name="esum")
                    et_tiles = []
                    for kt in range(ntile):
                        ssb, w = s_tiles[kt]
                        esb = ap.tile([128, 512], dt.float16, tag="esb", bufs=12,
                                      name="esb")
                        nc.scalar.activation(
                            esb[:, 0:w], ssb[:, 0:w], Act.Exp,
                            bias=negm[:], scale=1.0,
                            accum_out=esum[:, kt:kt + 1])
                        et_tiles.append((esb, w))
                    stot = ap.tile([128, 1], dt.float32, tag="stot", bufs=2,
                                   name="stot")
                    nc.vector.reduce_sum(stot[:], esum[:, 0:ntile], axis=Ax.X)
                    stot2 = ap.tile([128, 1], dt.float32, tag="stot2", bufs=2,
                                    name="stot2")
                    nc.scalar.mul(stot2[:], stot[:], float(np.sqrt(128.0)))
                    inv = ap.tile([128, 1], dt.float32, tag="inv", bufs=2,
                                  name="inv")
                    nc.vector.reciprocal(inv[:], stot2[:])
                    state[g] = (et_tiles, inv)

                def stage_z(g):
                    """Z accumulation, normalize, Z^T via DMA transpose."""
                    ntile = KT_TILES[g]
                    et_tiles, inv = state.pop(g)
                    zps = psa.tile([128, 512], dt.float32, tag="zps", bufs=2,
                                   name="zps")
                    nmm = sum(w // 128 for _, w in et_tiles)
                    i = 0
                    for kt, (esb, w) in enumerate(et_tiles):
                        for j in range(w // 128):
                            etp = psa.tile([128, 128], dt.float16, tag="etp",
                                           bufs=2, name="etp")
                            nc.tensor.transpose(
                                etp[:], esb[:, 128 * j:128 * j + 128], ident[:])
                            ets = ap.tile([128, 128], dt.float16, tag="ets",
                                          bufs=3, name="ets")
                            nc.vector.tensor_copy(ets[:], etp[:])
                            nc.tensor.matmul(zps[:], ets[:], V_sb[4 * kt + j][:],
                                             start=(i == 0), stop=(i == nmm - 1))
                            i += 1
                    zn = ap.tile([128, 512], dt.float16, tag="zn", bufs=3,
                                 name="zn")
                    nc.vector.tensor_scalar_mul(zn[:], zps[:], inv[:])
                    for j in range(4):
                        ztp = psa.tile([128, 128], dt.float16, tag="etp",
                                       bufs=2, name="ztp")
                        nc.tensor.transpose(ztp[:], zn[:, 128 * j:128 * j + 128],
                                            ident[:])
                        nc.vector.tensor_copy(
                            ZT_blk[g][:, 128 * j:128 * j + 128], ztp[:])

                def stage_o(g):
                    """Output projection for row block g (streamed per 512)."""
                    osb = op.tile([128, D], dt.float32, tag="osb", bufs=2,
                                  name="osb")
                    for dtile in range(4):
                        ops = psa.tile([128, 512], dt.float32, tag="ops", bufs=2,
                                       name="ops")
                        for h in range(4):
                            nc.tensor.matmul(
                                ops[:],
                                ZT_blk[g][:, 128 * h:128 * h + 128],
                                prj[h][:, 512 * dtile:512 * dtile + 512],
                                start=(h == 0), stop=(h == 3))
                        if dtile % 2 == 0:
                            nc.scalar.copy(
                                osb[:, 512 * dtile:512 * dtile + 512], ops[:])
                        else:
                            nc.vector.tensor_copy(
                                osb[:, 512 * dtile:512 * dtile + 512], ops[:])
                        nc.sync.dma_start(
                            out=OUT_d[128 * g:128 * g + 128,
                                      512 * dtile:512 * dtile + 512],
                            in_=osb[:, 512 * dtile:512 * dtile + 512])

                # software pipeline, 2-block lag: the E^T DMA transposes of
                # block g have ~2 iterations of PE work (s(g+1..2), z(g-1),
                # o(g-2)) to land before z(g) consumes them.
                stage_s(0)
                stage_e(0)
                stage_s(1)
                stage_e(1)
                for g in range(RB):
                    if g + 2 < RB:
                        stage_s(g + 2)
                        stage_e(g + 2)
                    stage_z(g)
                    if g >= 1:
                        stage_o(g - 1)
                stage_o(RB - 1)

    nc.compile()
    return nc


def host_prep(x, WQ, bQ, WK, bK, WV, bV, proj):
    """Collapse weights, transpose layouts, build per-core input maps."""
    x = np.ascontiguousarray(x, dtype=np.float32)
    WQc = WQ.reshape(4, HD, D).sum(0)
    bQc = bQ.reshape(4, HD).sum(0)
    projc = proj.reshape(4, HD, D).sum(0)

    WQT = np.ascontiguousarray(WQc.T).astype(np.float16)    # [D, HD]
    WKT = np.ascontiguousarray(WK.T).astype(np.float16)
    WVT = np.ascontiguousarray(WV.T).astype(np.float16)
    PRJ = np.ascontiguousarray(projc).astype(np.float16)    # [HD, D]
    bQc = np.ascontiguousarray(bQc.reshape(HD, 1), dtype=np.float32)
    bKc = np.ascontiguousarray(bK.reshape(HD, 1), dtype=np.float32)
    bVf = np.ascontiguousarray(
        np.broadcast_to(bV.reshape(1, HD), (128, HD))).astype(np.float16)
    idt = np.eye(128, dtype=np.float16)

    in_maps = []
    for core in range(NCORES):
        b, q = divmod(core, 2)
        xT = np.ascontiguousarray(x[b].T).astype(np.float16)    # [D, T]
        rows = np.concatenate(
            [np.arange(256 * g + 128 * q, 256 * g + 128 * q + 128)
             for g in range(RB)])
        xTq = np.ascontiguousarray(xT[:, rows])     # [D, 1024]
        msk = np.zeros((RB, 128, 512), dtype=np.float32)
        for g in range(RB):
            ntile = KT_TILES[g]
            base = 512 * (ntile - 1)                # keys covered by last tile
            key = base + np.arange(512)[None, :]
            row = (256 * g + 128 * q + np.arange(128))[:, None]
            msk[g] = np.where(key <= row, 0.0, NEG)
        in_maps.append({
            "xT": xT, "xTq": xTq, "WKT": WKT, "WVT": WVT, "WQT": WQT,
            "PRJ": PRJ, "bK": bKc, "bQ": bQc, "BVF": bVf,
            "IDT": idt, "MSK": msk,
        })
    return in_maps


def assemble(results):
    """Gather per-core [1024, D] outputs into [B, T, D]."""
    y = np.empty((B, T, D), dtype=np.float32)
    for core in range(NCORES):
        b, q = divmod(core, 2)
        o = results[core]["out"]
        for g in range(RB):
            y[b, 256 * g + 128 * q:256 * g + 128 * q + 128] = \
                o[128 * g:128 * g + 128]
    return y


_NC_CACHE = None


def kernel(x, WQ, bQ, WK, bK, WV, bV, proj):
    global _NC_CACHE
    in_maps = host_prep(np.asarray(x), np.asarray(WQ), np.asarray(bQ),
                        np.asarray(WK), np.asarray(bK), np.asarray(WV),
                        np.asarray(bV), np.asarray(proj))
    if _NC_CACHE is None:
        _NC_CACHE = build_kernel()
    res = run_bass_kernel_spmd(_NC_CACHE, in_maps, list(range(NCORES)))
    return assemble(res.results)


# revision 20
# speedup vs baseline: 1.1801x; 1.1801x over previous
"""GroupedQueryAttention Trainium2 Bass kernel (8 NeuronCores, SPMD), v2.

Same collapsed-GQA math as v1 (see kernel.py docstring): the reference's
tiled K/V + full-2048-dim attention reduces to one 512-dim attention with
summed WQ row-blocks and summed proj row-blocks.

v2 changes vs v1 (290us -> ~249us):
- fp16 matmul operands everywhere (x, weights, Q/K/V, E, Z, proj).
  PSUM accumulation stays fp32; softmax scalars stay fp32.  Host-side
  numerics check: rel err 3.1e-3 (same as fp32r) << 2e-2 tolerance.
  Halves DMA traffic and SBUF footprint; E^T/Z^T PE transposes run at
  1 cycle/row instead of 2.
- Output projection interleaved into the attention software pipeline
  (o(g-1) emitted after z(g)) to remove the attention->projection
  transition bubble; 2-block software-pipeline lag keeps PE fed.
- Chunk-granular causality: even row blocks compute their last key tile
  at width 256 (the upper 256 keys are fully masked anyway).
- K PSUM drains on scalar (bias add) run in parallel with V drains on
  vector; output DMA streamed per 512-column chunk.
- First compute-feeding DMAs (wk0/wv0/xt0) issued before misc constant
  loads to shrink the kernel lead-in.
(DMA XBAR transposes and remote_dma pair-exchange of K/V were tried and
rejected: the former is racy/wrong-output-prone here, the latter's
SWDGE desc-gen ucode faults on this runtime.)

Sharding unchanged: 8 cores = 4 batches x 2 interleaved 128-row blocks;
causal key tiles per block [1,1,2,2,3,3,4,4] identical on both pair
cores -> single SPMD program.
"""

import numpy as np

import concourse.bacc as bacc
import concourse.mybir as mybir
from concourse.tile import TileContext
from concourse.bass_utils import run_bass_kernel_spmd

B, T, D = 4, 2048, 2048
HD = 512                 # collapsed head dim
NCORES = 8
RB = 8                   # 128-row blocks per core
KT_TILES = [1, 1, 2, 2, 3, 3, 4, 4]   # causal 512-wide key tiles per block
DCH = D // 128           # 16 contraction chunks
dt = mybir.dt
NEG = -30000.0           # fp16-safe mask value


def build_kernel():
    nc = bacc.Bacc(None, target_bir_lowering=False)

    xT_d = nc.dram_tensor("xT", [D, T], dt.float16, kind="ExternalInput")
    xTq_d = nc.dram_tensor("xTq", [D, 1024], dt.float16, kind="ExternalInput")
    WKT_d = nc.dram_tensor("WKT", [D, HD], dt.float16, kind="ExternalInput")
    WVT_d = nc.dram_tensor("WVT", [D, HD], dt.float16, kind="ExternalInput")
    WQT_d = nc.dram_tensor("WQT", [D, HD], dt.float16, kind="ExternalInput")
    PRJ_d = nc.dram_tensor("PRJ", [HD, D], dt.float16, kind="ExternalInput")
    bK_d = nc.dram_tensor("bK", [HD, 1], dt.float32, kind="ExternalInput")
    bQ_d = nc.dram_tensor("bQ", [HD, 1], dt.float32, kind="ExternalInput")
    BVF_d = nc.dram_tensor("BVF", [128, HD], dt.float16, kind="ExternalInput")
    IDT_d = nc.dram_tensor("IDT", [128, 128], dt.float16, kind="ExternalInput")
    MSK_d = nc.dram_tensor("MSK", [RB, 128, 512], dt.float32, kind="ExternalInput")
    OUT_d = nc.dram_tensor("out", [1024, D], dt.float32, kind="ExternalOutput")

    Act = mybir.ActivationFunctionType
    Ax = mybir.AxisListType

    with TileContext(nc) as tc:
        with tc.tile_pool(name="persist", bufs=1) as pp:
            # ---- persistent tiles ------------------------------------------
            KT_sb = [pp.tile([128, T], dt.float16, tag=f"KT{h}", name=f"KT{h}")
                     for h in range(4)]
            V_sb = [pp.tile([128, HD], dt.float16, tag=f"V{k}", name=f"V{k}")
                    for k in range(16)]
            QT_sb = [pp.tile([128, 1024], dt.float16, tag=f"QT{h}", name=f"QT{h}")
                     for h in range(4)]
            # Z^T per row block, chunk-transposed: hd-chunk h at columns
            # 128*h. One contiguous [128,512] tile per block — XBAR DMA
            # transpose needs a contiguous destination (strided slices of a
            # wider tile produce wrong output on hardware).
            ZT_blk = [pp.tile([128, 512], dt.float16, tag=f"ZTb{g}",
                              name=f"ZTb{g}") for g in range(RB)]
            bvf = pp.tile([128, HD], dt.float16, tag="bvf")
            ident = pp.tile([128, 128], dt.float16, tag="ident")

            # ---- stage P: projections --------------------------------------
            # x^T streamed by 128-D chunk; weights streamed with the first key
            # group then resident. Per 512-key group: 4 PSUM banks accumulate
            # K^T hd-tiles ([hd, keys], bias per partition) and 4 banks
            # accumulate V key-blocks directly in [keys, hd] layout (xt chunk
            # as lhsT); V bias enters as a rank-1 ones x bV matmul.
            with tc.tile_pool(name="wpool", bufs=1) as wp, \
                 tc.tile_pool(name="xstream", bufs=6) as xp, \
                 tc.tile_pool(name="psP", bufs=1, space="PSUM") as psp:
                wk = [None] * DCH
                wv = [None] * DCH
                for rt in range(4):                      # key 512-col groups
                    kps = [psp.tile([128, 512], dt.float32, tag=f"kps{h}",
                                    name=f"kps{h}") for h in range(4)]
                    vps = [psp.tile([128, 512], dt.float32, tag=f"vps{j}",
                                    name=f"vps{j}") for j in range(4)]
                    for c in range(DCH):
                        xt = xp.tile([128, 512], dt.float16, tag="xs", name="xt")
                        xeng = nc.sync if c % 2 == 0 else nc.scalar
                        xeng.dma_start(
                            out=xt[:],
                            in_=xT_d[128 * c:128 * c + 128, 512 * rt:512 * rt + 512])
                        if rt == 0:   # stream weights in with the first pass
                            tk = wp.tile([128, HD], dt.float16, tag=f"wk{c}",
                                         name=f"wk{c}")
                            nc.sync.dma_start(
                                out=tk[:], in_=WKT_d[128 * c:128 * c + 128, :])
                            wk[c] = tk
                            tv = wp.tile([128, HD], dt.float16, tag=f"wv{c}",
                                         name=f"wv{c}")
                            nc.scalar.dma_start(
                                out=tv[:], in_=WVT_d[128 * c:128 * c + 128, :])
                            wv[c] = tv
                        st = (c == 0)
                        for h in range(4):
                            nc.tensor.matmul(kps[h][:],
                                             wk[c][:, 128 * h:128 * h + 128],
                                             xt[:], start=st, stop=(c == DCH - 1))
                        for j in range(4):
                            nc.tensor.matmul(vps[j][:],
                                             xt[:, 128 * j:128 * j + 128],
                                             wv[c][:], start=st,
                                             stop=(c == DCH - 1))
                    if rt == 0:
                        nc.scalar.dma_start(out=bvf[:], in_=BVF_d[:])
                        nc.scalar.dma_start(out=ident[:], in_=IDT_d[:])
                        bias_sb = {}
                        for nm, src in (("bK", bK_d), ("bQ", bQ_d)):
                            for h in range(4):
                                t = pp.tile([128, 1], dt.float32, tag=f"{nm}{h}",
                                            name=f"{nm}{h}")
                                nc.sync.dma_start(
                                    out=t[:], in_=src[128 * h:128 * h + 128, :])
                                bias_sb[nm, h] = t
                    # drain K on scalar (bias add) and V on vector, in
                    # parallel, so the next group's matmuls get their PSUM
                    # banks back ~2x sooner
                    for h in range(4):
                        nc.scalar.activation(
                            KT_sb[h][:, 512 * rt:512 * rt + 512], kps[h][:],
                            Act.Identity, bias=bias_sb["bK", h][:], scale=1.0)
                    for j in range(4):
                        nc.vector.tensor_add(V_sb[4 * rt + j][:], vps[j][:],
                                             bvf[:])
                # Qc^T for this core's 1024 rows (reuses the wk slots).
                wq = []
                for c in range(DCH):
                    tq = wp.tile([128, HD], dt.float16, tag=f"wk{c}", name=f"wq{c}")
                    nc.scalar.dma_start(out=tq[:], in_=WQT_d[128 * c:128 * c + 128, :])
                    wq.append(tq)
                for rt in range(2):
                    qps = [psp.tile([128, 512], dt.float32, tag=f"kps{h}",
                                    name=f"qps{h}") for h in range(4)]
                    for c in range(DCH):
                        xt = xp.tile([128, 512], dt.float16, tag="xs", name="xt")
                        xeng = nc.sync if c % 2 == 0 else nc.scalar
                        xeng.dma_start(
                            out=xt[:],
                            in_=xTq_d[128 * c:128 * c + 128, 512 * rt:512 * rt + 512])
                        for h in range(4):
                            nc.tensor.matmul(qps[h][:],
                                             wq[c][:, 128 * h:128 * h + 128],
                                             xt[:], start=(c == 0),
                                             stop=(c == DCH - 1))
                    for h in range(4):
                        nc.scalar.activation(
                            QT_sb[h][:, 512 * rt:512 * rt + 512], qps[h][:],
                            Act.Identity, bias=bias_sb["bQ", h][:], scale=1.0)

            # ---- stages A+O: attention with interleaved output proj --------
            with tc.tile_pool(name="attn", bufs=1) as ap, \
                 tc.tile_pool(name="oproj", bufs=1) as op, \
                 tc.tile_pool(name="psA", bufs=1, space="PSUM") as psa:
                # prefetch output-projection weights while attention runs
                prj = []
                for h in range(4):
                    t = op.tile([128, D], dt.float16, tag=f"prj{h}", name=f"prj{h}")
                    nc.sync.dma_start(out=t[:], in_=PRJ_d[128 * h:128 * h + 128, :])
                    prj.append(t)

                state = {}

                def stage_s(g):
                    """S matmuls + mask + per-tile max for row block g.

                    Rows of block g (either pair core) only reach key
                    128*(2g+2)-1, so the last 512-key tile is computed at
                    width 256 for even g (its upper 256 keys are entirely
                    masked) — chunk-granular causality.
                    """
                    ntile = KT_TILES[g]
                    lw = 256 if g % 2 == 0 else 512   # last-tile width
                    mpart = ap.tile([128, 4], dt.float32, tag="mpart", bufs=2,
                                    name="mpart")
                    s_tiles = []
                    for kt in range(ntile):
                        w = lw if kt == ntile - 1 else 512
                        sps = psa.tile([128, 512], dt.float32, tag="sps", bufs=2,
                                       name="sps")
                        for h in range(4):
                            nc.tensor.matmul(
                                sps[:, 0:w], QT_sb[h][:, 128 * g:128 * g + 128],
                                KT_sb[h][:, 512 * kt:512 * kt + w],
                                start=(h == 0), stop=(h == 3))
                        ssb = ap.tile([128, 512], dt.float32, tag="ssb", bufs=9,
                                      name="ssb")
                        if kt == ntile - 1:
                            mt = ap.tile([128, 512], dt.float32, tag="mask",
                                         bufs=2, name="mt")
                            nc.sync.dma_start(out=mt[:, 0:w], in_=MSK_d[g, :, 0:w])
                            nc.vector.tensor_add(ssb[:, 0:w], sps[:, 0:w],
                                                 mt[:, 0:w])
                        else:
                            nc.scalar.copy(ssb[:, 0:w], sps[:, 0:w])
                        nc.vector.reduce_max(mpart[:, kt:kt + 1], ssb[:, 0:w],
                                             axis=Ax.X)
                        s_tiles.append((ssb, w))
                    state[g] = (s_tiles, mpart)

                def stage_e(g):
                    """negmax + exp + E^T DMA transposes + 1/(sum*sqrt(hs))."""
                    ntile = KT_TILES[g]
                    s_tiles, mpart = state[g]
                    negm = ap.tile([128, 1], dt.float32, tag="negm", bufs=2,
                                   name="negm")
                    nc.vector.reduce_max(negm[:], mpart[:, 0:ntile], axis=Ax.X,
                                         negate=True)
                    esum = ap.tile([128, 4], dt.float32, tag="esum", bufs=2,
                                   name="esum")
                    et_tiles = []
                    for kt in range(ntile):
                        ssb, w = s_tiles[kt]
                        esb = ap.tile([128, 512], dt.float16, tag="esb", bufs=12,
                                      name="esb")
                        nc.scalar.activation(
                            esb[:, 0:w], ssb[:, 0:w], Act.Exp,
                            bias=negm[:], scale=1.0,
                            accum_out=esum[:, kt:kt + 1])
                        et_tiles.append((esb, w))
                    stot = ap.tile([128, 1], dt.float32, tag="stot", bufs=2,
                                   name="stot")
                    nc.vector.reduce_sum(stot[:], esum[:, 0:ntile], axis=Ax.X)
                    stot2 = ap.tile([128, 1], dt.float32, tag="stot2", bufs=2,
                                    name="stot2")
                    nc.scalar.mul(stot2[:], stot[:], float(np.sqrt(128.0)))
                    inv = ap.tile([128, 1], dt.float32, tag="inv", bufs=2,
                                  name="inv")
                    nc.vector.reciprocal(inv[:], stot2[:])
                    state[g] = (et_tiles, inv)

                def stage_z(g):
                    """Z accumulation, normalize, Z^T via DMA transpose."""
                    ntile = KT_TILES[g]
                    et_tiles, inv = state.pop(g)
                    zps = psa.tile([128, 512], dt.float32, tag="zps", bufs=2,
                                   name="zps")
                    nmm = sum(w // 128 for _, w in et_tiles)
                    i = 0
                    for kt, (esb, w) in enumerate(et_tiles):
                        for j in range(w // 128):
                            etp = psa.tile([128, 128], dt.float16, tag="etp",
                                           bufs=2, name="etp")
                            nc.tensor.transpose(
                                etp[:], esb[:, 128 * j:128 * j + 128], ident[:])
                            ets = ap.tile([128, 128], dt.float16, tag="ets",
                                          bufs=3, name="ets")
                            nc.vector.tensor_copy(ets[:], etp[:])
                            nc.tensor.matmul(zps[:], ets[:], V_sb[4 * kt + j][:],
                                             start=(i == 0), stop=(i == nmm - 1))
                            i += 1
                    zn = ap.tile([128, 512], dt.float16, tag="zn", bufs=3,
                                 name="zn")
                    nc.vector.tensor_scalar_mul(zn[:], zps[:], inv[:])
                    for j in range(4):
                        ztp = psa.tile([128, 128], dt.float16, tag="etp",
                                       bufs=2, name="ztp")
                        nc.tensor.transpose(ztp[:], zn[:, 128 * j:128 * j + 128],
                                            ident[:])
                        nc.vector.tensor_copy(
                            ZT_blk[g][:, 128 * j:128 * j + 128], ztp[:])

                def stage_o(g):
                    """Output projection for row block g (streamed per 512)."""
                    osb = op.tile([128, D], dt.float32, tag="osb", bufs=2,
                                  name="osb")
                    for dtile in range(4):
                        ops = psa.tile([128, 512], dt.float32, tag="ops", bufs=2,
                                       name="ops")
                        for h in range(4):
                            nc.tensor.matmul(
                                ops[:],
                                ZT_blk[g][:, 128 * h:128 * h + 128],
                                prj[h][:, 512 * dtile:512 * dtile + 512],
                                start=(h == 0), stop=(h == 3))
                        nc.scalar.copy(osb[:, 512 * dtile:512 * dtile + 512],
                                       ops[:])
                        nc.sync.dma_start(
                            out=OUT_d[128 * g:128 * g + 128,
                                      512 * dtile:512 * dtile + 512],
                            in_=osb[:, 512 * dtile:512 * dtile + 512])

                # software pipeline, 2-block lag: the E^T DMA transposes of
                # block g have ~2 iterations of PE work (s(g+1..2), z(g-1),
                # o(g-2)) to land before z(g) consumes them.
                stage_s(0)
                stage_e(0)
                stage_s(1)
                stage_e(1)
                for g in range(RB):
                    if g + 2 < RB:
                        stage_s(g + 2)
                        stage_e(g + 2)
                    stage_z(g)
                    if g >= 1:
                        stage_o(g - 1)
                stage_o(RB - 1)

    nc.compile()
    return nc


def host_prep(x, WQ, bQ, WK, bK, WV, bV, proj):
    """Collapse weights, transpose layouts, build per-core input maps."""
    x = np.ascontiguousarray(x, dtype=np.float32)
    WQc = WQ.reshape(4, HD, D).sum(0)
    bQc = bQ.reshape(4, HD).sum(0)
    projc = proj.reshape(4, HD, D).sum(0)

    WQT = np.ascontiguousarray(WQc.T).astype(np.float16)    # [D, HD]
    WKT = np.ascontiguousarray(WK.T).astype(np.float16)
    WVT = np.ascontiguousarray(WV.T).astype(np.float16)
    PRJ = np.ascontiguousarray(projc).astype(np.float16)    # [HD, D]
    bQc = np.ascontiguousarray(bQc.reshape(HD, 1), dtype=np.float32)
    bKc = np.ascontiguousarray(bK.reshape(HD, 1), dtype=np.float32)
    bVf = np.ascontiguousarray(
        np.broadcast_to(bV.reshape(1, HD), (128, HD))).astype(np.float16)
    idt = np.eye(128, dtype=np.float16)

    in_maps = []
    for core in range(NCORES):
        b, q = divmod(core, 2)
        xT = np.ascontiguousarray(x[b].T).astype(np.float16)    # [D, T]
        rows = np.concatenate(
            [np.arange(256 * g + 128 * q, 256 * g + 128 * q + 128)
             for g in range(RB)])
        xTq = np.ascontiguousarray(xT[:, rows])     # [D, 1024]
        msk = np.zeros((RB, 128, 512), dtype=np.float32)
        for g in range(RB):
            ntile = KT_TILES[g]
            base = 512 * (ntile - 1)                # keys covered by last tile
            key = base + np.arange(512)[None, :]
            row = (256 * g + 128 * q + np.arange(128))[:, None]
            msk[g] = np.where(key <= row, 0.0, NEG)
        in_maps.append({
            "xT": xT, "xTq": xTq, "WKT": WKT, "WVT": WVT, "WQT": WQT,
            "PRJ": PRJ, "bK": bKc, "bQ": bQc, "BVF": bVf,
            "IDT": idt, "MSK": msk,
        })
    return in_maps


def assemble(results):
    """Gather per-core [1024, D] outputs into [B, T, D]."""
    y = np.empty((B, T, D), dtype=np.float32)
    for core in range(NCORES):
        b, q = divmod(core, 2)
        o = results[core]["out"]
        for g in range(RB):
            y[b, 256 * g + 128 * q:256 * g + 128 * q + 128] = \
                o[128 * g:128 * g + 128]
    return y


_NC_CACHE = None


def kernel(x, WQ, bQ, WK, bK, WV, bV, proj):
    global _NC_CACHE
    in_maps = host_prep(np.asarray(x), np.asarray(WQ), np.asarray(bQ),
                        np.asarray(WK), np.asarray(bK), np.asarray(WV),
                        np.asarray(bV), np.asarray(proj))
    if _NC_CACHE is None:
        _NC_CACHE = build_kernel()
    res = run_bass_kernel_spmd(_NC_CACHE, in_maps, list(range(NCORES)))
    return assemble(res.results)


# revision 21
# speedup vs baseline: 1.1885x; 1.0071x over previous
"""GroupedQueryAttention Trainium2 Bass kernel (8 NeuronCores, SPMD), v2.

Same collapsed-GQA math as v1 (see kernel.py docstring): the reference's
tiled K/V + full-2048-dim attention reduces to one 512-dim attention with
summed WQ row-blocks and summed proj row-blocks.

v2 changes vs v1 (290us -> ~249us):
- fp16 matmul operands everywhere (x, weights, Q/K/V, E, Z, proj).
  PSUM accumulation stays fp32; softmax scalars stay fp32.  Host-side
  numerics check: rel err 3.1e-3 (same as fp32r) << 2e-2 tolerance.
  Halves DMA traffic and SBUF footprint; E^T/Z^T PE transposes run at
  1 cycle/row instead of 2.
- Output projection interleaved into the attention software pipeline
  (o(g-1) emitted after z(g)) to remove the attention->projection
  transition bubble; 2-block software-pipeline lag keeps PE fed.
- Chunk-granular causality: even row blocks compute their last key tile
  at width 256 (the upper 256 keys are fully masked anyway).
- K PSUM drains on scalar (bias add) run in parallel with V drains on
  vector; output DMA streamed per 512-column chunk.
- First compute-feeding DMAs (wk0/wv0/xt0) issued before misc constant
  loads to shrink the kernel lead-in.
(DMA XBAR transposes and remote_dma pair-exchange of K/V were tried and
rejected: the former is racy/wrong-output-prone here, the latter's
SWDGE desc-gen ucode faults on this runtime.)

Sharding unchanged: 8 cores = 4 batches x 2 interleaved 128-row blocks;
causal key tiles per block [1,1,2,2,3,3,4,4] identical on both pair
cores -> single SPMD program.
"""

import numpy as np

import concourse.bacc as bacc
import concourse.mybir as mybir
from concourse.tile import TileContext
from concourse.bass_utils import run_bass_kernel_spmd

B, T, D = 4, 2048, 2048
HD = 512                 # collapsed head dim
NCORES = 8
RB = 8                   # 128-row blocks per core
KT_TILES = [1, 1, 2, 2, 3, 3, 4, 4]   # causal 512-wide key tiles per block
DCH = D // 128           # 16 contraction chunks
dt = mybir.dt
NEG = -30000.0           # fp16-safe mask value


def build_kernel():
    nc = bacc.Bacc(None, target_bir_lowering=False)

    xT_d = nc.dram_tensor("xT", [D, T], dt.float16, kind="ExternalInput")
    xTq_d = nc.dram_tensor("xTq", [D, 1024], dt.float16, kind="ExternalInput")
    WKT_d = nc.dram_tensor("WKT", [D, HD], dt.float16, kind="ExternalInput")
    WVT_d = nc.dram_tensor("WVT", [D, HD], dt.float16, kind="ExternalInput")
    WQT_d = nc.dram_tensor("WQT", [D, HD], dt.float16, kind="ExternalInput")
    PRJ_d = nc.dram_tensor("PRJ", [HD, D], dt.float16, kind="ExternalInput")
    bK_d = nc.dram_tensor("bK", [HD, 1], dt.float32, kind="ExternalInput")
    bQ_d = nc.dram_tensor("bQ", [HD, 1], dt.float32, kind="ExternalInput")
    BVF_d = nc.dram_tensor("BVF", [128, HD], dt.float16, kind="ExternalInput")
    IDT_d = nc.dram_tensor("IDT", [128, 128], dt.float16, kind="ExternalInput")
    MSK_d = nc.dram_tensor("MSK", [RB, 128, 512], dt.float32, kind="ExternalInput")
    OUT_d = nc.dram_tensor("out", [1024, D], dt.float32, kind="ExternalOutput")

    Act = mybir.ActivationFunctionType
    Ax = mybir.AxisListType

    with TileContext(nc) as tc:
        with tc.tile_pool(name="persist", bufs=1) as pp:
            # ---- persistent tiles ------------------------------------------
            KT_sb = [pp.tile([128, T], dt.float16, tag=f"KT{h}", name=f"KT{h}")
                     for h in range(4)]
            V_sb = [pp.tile([128, HD], dt.float16, tag=f"V{k}", name=f"V{k}")
                    for k in range(16)]
            QT_sb = [pp.tile([128, 1024], dt.float16, tag=f"QT{h}", name=f"QT{h}")
                     for h in range(4)]
            # Z^T per row block, chunk-transposed: hd-chunk h at columns
            # 128*h. One contiguous [128,512] tile per block — XBAR DMA
            # transpose needs a contiguous destination (strided slices of a
            # wider tile produce wrong output on hardware).
            ZT_blk = [pp.tile([128, 512], dt.float16, tag=f"ZTb{g}",
                              name=f"ZTb{g}") for g in range(RB)]
            bvf = pp.tile([128, HD], dt.float16, tag="bvf")
            ident = pp.tile([128, 128], dt.float16, tag="ident")

            # ---- stage P: projections --------------------------------------
            # x^T streamed by 128-D chunk; weights streamed with the first key
            # group then resident. Per 512-key group: 4 PSUM banks accumulate
            # K^T hd-tiles ([hd, keys], bias per partition) and 4 banks
            # accumulate V key-blocks directly in [keys, hd] layout (xt chunk
            # as lhsT); V bias enters as a rank-1 ones x bV matmul.
            with tc.tile_pool(name="wpool", bufs=1) as wp, \
                 tc.tile_pool(name="xstream", bufs=6) as xp, \
                 tc.tile_pool(name="psP", bufs=1, space="PSUM") as psp:
                wk = [None] * DCH
                wv = [None] * DCH
                for rt in range(4):                      # key 512-col groups
                    kps = [psp.tile([128, 512], dt.float32, tag=f"kps{h}",
                                    name=f"kps{h}") for h in range(4)]
                    vps = [psp.tile([128, 512], dt.float32, tag=f"vps{j}",
                                    name=f"vps{j}") for j in range(4)]
                    for c in range(DCH):
                        xt = xp.tile([128, 512], dt.float16, tag="xs", name="xt")
                        if rt == 0 and c == 0:
                            # first chunk split across both HWDGE queues so
                            # the very first matmul's input lands sooner
                            nc.sync.dma_start(
                                out=xt[:, 0:256], in_=xT_d[0:128, 0:256])
                            nc.scalar.dma_start(
                                out=xt[:, 256:512], in_=xT_d[0:128, 256:512])
                        else:
                            xeng = nc.sync if c % 2 == 0 else nc.scalar
                            xeng.dma_start(
                                out=xt[:],
                                in_=xT_d[128 * c:128 * c + 128,
                                         512 * rt:512 * rt + 512])
                        if rt == 0:   # stream weights in with the first pass
                            tk = wp.tile([128, HD], dt.float16, tag=f"wk{c}",
                                         name=f"wk{c}")
                            nc.sync.dma_start(
                                out=tk[:], in_=WKT_d[128 * c:128 * c + 128, :])
                            wk[c] = tk
                            tv = wp.tile([128, HD], dt.float16, tag=f"wv{c}",
                                         name=f"wv{c}")
                            nc.scalar.dma_start(
                                out=tv[:], in_=WVT_d[128 * c:128 * c + 128, :])
                            wv[c] = tv
                        st = (c == 0)
                        for h in range(4):
                            nc.tensor.matmul(kps[h][:],
                                             wk[c][:, 128 * h:128 * h + 128],
                                             xt[:], start=st, stop=(c == DCH - 1))
                        for j in range(4):
                            nc.tensor.matmul(vps[j][:],
                                             xt[:, 128 * j:128 * j + 128],
                                             wv[c][:], start=st,
                                             stop=(c == DCH - 1))
                    if rt == 0:
                        nc.scalar.dma_start(out=bvf[:], in_=BVF_d[:])
                        nc.scalar.dma_start(out=ident[:], in_=IDT_d[:])
                        bias_sb = {}
                        for nm, src in (("bK", bK_d), ("bQ", bQ_d)):
                            for h in range(4):
                                t = pp.tile([128, 1], dt.float32, tag=f"{nm}{h}",
                                            name=f"{nm}{h}")
                                nc.sync.dma_start(
                                    out=t[:], in_=src[128 * h:128 * h + 128, :])
                                bias_sb[nm, h] = t
                    # drain K on scalar (bias add) and V on vector, in
                    # parallel, so the next group's matmuls get their PSUM
                    # banks back ~2x sooner
                    for h in range(4):
                        nc.scalar.activation(
                            KT_sb[h][:, 512 * rt:512 * rt + 512], kps[h][:],
                            Act.Identity, bias=bias_sb["bK", h][:], scale=1.0)
                    for j in range(4):
                        nc.vector.tensor_add(V_sb[4 * rt + j][:], vps[j][:],
                                             bvf[:])
                # Qc^T for this core's 1024 rows (reuses the wk slots).
                wq = []
                for c in range(DCH):
                    tq = wp.tile([128, HD], dt.float16, tag=f"wk{c}", name=f"wq{c}")
                    nc.scalar.dma_start(out=tq[:], in_=WQT_d[128 * c:128 * c + 128, :])
                    wq.append(tq)
                for rt in range(2):
                    qps = [psp.tile([128, 512], dt.float32, tag=f"kps{h}",
                                    name=f"qps{h}") for h in range(4)]
                    for c in range(DCH):
                        xt = xp.tile([128, 512], dt.float16, tag="xs", name="xt")
                        xeng = nc.sync if c % 2 == 0 else nc.scalar
                        xeng.dma_start(
                            out=xt[:],
                            in_=xTq_d[128 * c:128 * c + 128, 512 * rt:512 * rt + 512])
                        for h in range(4):
                            nc.tensor.matmul(qps[h][:],
                                             wq[c][:, 128 * h:128 * h + 128],
                                             xt[:], start=(c == 0),
                                             stop=(c == DCH - 1))
                    for h in range(4):
                        nc.scalar.activation(
                            QT_sb[h][:, 512 * rt:512 * rt + 512], qps[h][:],
                            Act.Identity, bias=bias_sb["bQ", h][:], scale=1.0)

            # ---- stages A+O: attention with interleaved output proj --------
            with tc.tile_pool(name="attn", bufs=1) as ap, \
                 tc.tile_pool(name="oproj", bufs=1) as op, \
                 tc.tile_pool(name="psA", bufs=1, space="PSUM") as psa:
                # prefetch output-projection weights while attention runs
                prj = []
                for h in range(4):
                    t = op.tile([128, D], dt.float16, tag=f"prj{h}", name=f"prj{h}")
                    nc.sync.dma_start(out=t[:], in_=PRJ_d[128 * h:128 * h + 128, :])
                    prj.append(t)

                state = {}

                def stage_s(g):
                    """S matmuls + mask + per-tile max for row block g.

                    Rows of block g (either pair core) only reach key
                    128*(2g+2)-1, so the last 512-key tile is computed at
                    width 256 for even g (its upper 256 keys are entirely
                    masked) — chunk-granular causality.
                    """
                    ntile = KT_TILES[g]
                    lw = 256 if g % 2 == 0 else 512   # last-tile width
                    mpart = ap.tile([128, 4], dt.float32, tag="mpart", bufs=2,
                                    name="mpart")
                    s_tiles = []
                    for kt in range(ntile):
                        w = lw if kt == ntile - 1 else 512
                        sps = psa.tile([128, 512], dt.float32, tag="sps", bufs=2,
                                       name="sps")
                        for h in range(4):
                            nc.tensor.matmul(
                                sps[:, 0:w], QT_sb[h][:, 128 * g:128 * g + 128],
                                KT_sb[h][:, 512 * kt:512 * kt + w],
                                start=(h == 0), stop=(h == 3))
                        ssb = ap.tile([128, 512], dt.float32, tag="ssb", bufs=9,
                                      name="ssb")
                        if kt == ntile - 1:
                            mt = ap.tile([128, 512], dt.float32, tag="mask",
                                         bufs=2, name="mt")
                            nc.sync.dma_start(out=mt[:, 0:w], in_=MSK_d[g, :, 0:w])
                            nc.vector.tensor_add(ssb[:, 0:w], sps[:, 0:w],
                                                 mt[:, 0:w])
                        else:
                            nc.scalar.copy(ssb[:, 0:w], sps[:, 0:w])
                        nc.vector.reduce_max(mpart[:, kt:kt + 1], ssb[:, 0:w],
                                             axis=Ax.X)
                        s_tiles.append((ssb, w))
                    state[g] = (s_tiles, mpart)

                def stage_e(g):
                    """negmax + exp + E^T DMA transposes + 1/(sum*sqrt(hs))."""
                    ntile = KT_TILES[g]
                    s_tiles, mpart = state[g]
                    negm = ap.tile([128, 1], dt.float32, tag="negm", bufs=2,
                                   name="negm")
                    nc.vector.reduce_max(negm[:], mpart[:, 0:ntile], axis=Ax.X,
                                         negate=True)
                    esum = ap.tile([128, 4], dt.float32, tag="esum", bufs=2,
                                   name="esum")
                    et_tiles = []
                    for kt in range(ntile):
                        ssb, w = s_tiles[kt]
                        esb = ap.tile([128, 512], dt.float16, tag="esb", bufs=12,
                                      name="esb")
                        nc.scalar.activation(
                            esb[:, 0:w], ssb[:, 0:w], Act.Exp,
                            bias=negm[:], scale=1.0,
                            accum_out=esum[:, kt:kt + 1])
                        et_tiles.append((esb, w))
                    stot = ap.tile([128, 1], dt.float32, tag="stot", bufs=2,
                                   name="stot")
                    nc.vector.reduce_sum(stot[:], esum[:, 0:ntile], axis=Ax.X)
                    stot2 = ap.tile([128, 1], dt.float32, tag="stot2", bufs=2,
                                    name="stot2")
                    nc.scalar.mul(stot2[:], stot[:], float(np.sqrt(128.0)))
                    inv = ap.tile([128, 1], dt.float32, tag="inv", bufs=2,
                                  name="inv")
                    nc.vector.reciprocal(inv[:], stot2[:])
                    state[g] = (et_tiles, inv)

                def stage_z(g):
                    """Z accumulation, normalize, Z^T via DMA transpose."""
                    ntile = KT_TILES[g]
                    et_tiles, inv = state.pop(g)
                    zps = psa.tile([128, 512], dt.float32, tag="zps", bufs=2,
                                   name="zps")
                    nmm = sum(w // 128 for _, w in et_tiles)
                    i = 0
                    for kt, (esb, w) in enumerate(et_tiles):
                        for j in range(w // 128):
                            etp = psa.tile([128, 128], dt.float16, tag="etp",
                                           bufs=2, name="etp")
                            nc.tensor.transpose(
                                etp[:], esb[:, 128 * j:128 * j + 128], ident[:])
                            ets = ap.tile([128, 128], dt.float16, tag="ets",
                                          bufs=3, name="ets")
                            nc.vector.tensor_copy(ets[:], etp[:])
                            nc.tensor.matmul(zps[:], ets[:], V_sb[4 * kt + j][:],
                                             start=(i == 0), stop=(i == nmm - 1))
                            i += 1
                    zn = ap.tile([128, 512], dt.float16, tag="zn", bufs=3,
                                 name="zn")
                    nc.vector.tensor_scalar_mul(zn[:], zps[:], inv[:])
                    for j in range(4):
                        ztp = psa.tile([128, 128], dt.float16, tag="etp",
                                       bufs=2, name="ztp")
                        nc.tensor.transpose(ztp[:], zn[:, 128 * j:128 * j + 128],
                                            ident[:])
                        nc.vector.tensor_copy(
                            ZT_blk[g][:, 128 * j:128 * j + 128], ztp[:])

                def stage_o(g):
                    """Output projection for row block g (streamed per 512)."""
                    osb = op.tile([128, D], dt.float32, tag="osb", bufs=2,
                                  name="osb")
                    for dtile in range(4):
                        ops = psa.tile([128, 512], dt.float32, tag="ops", bufs=2,
                                       name="ops")
                        for h in range(4):
                            nc.tensor.matmul(
                                ops[:],
                                ZT_blk[g][:, 128 * h:128 * h + 128],
                                prj[h][:, 512 * dtile:512 * dtile + 512],
                                start=(h == 0), stop=(h == 3))
                        nc.scalar.copy(osb[:, 512 * dtile:512 * dtile + 512],
                                       ops[:])
                        nc.sync.dma_start(
                            out=OUT_d[128 * g:128 * g + 128,
                                      512 * dtile:512 * dtile + 512],
                            in_=osb[:, 512 * dtile:512 * dtile + 512])

                # software pipeline, 2-block lag, largest blocks first:
                # big early blocks give the softmax/transpose chain more PE
                # work to hide under, and the kernel tail ends on the
                # smallest block's Z + O.
                order = list(range(RB - 1, -1, -1))
                stage_s(order[0])
                stage_e(order[0])
                stage_s(order[1])
                stage_e(order[1])
                for i, g in enumerate(order):
                    if i + 2 < RB:
                        stage_s(order[i + 2])
                        stage_e(order[i + 2])
                    stage_z(g)
                    if i >= 1:
                        stage_o(order[i - 1])
                stage_o(order[-1])

    nc.compile()
    return nc


def host_prep(x, WQ, bQ, WK, bK, WV, bV, proj):
    """Collapse weights, transpose layouts, build per-core input maps."""
    x = np.ascontiguousarray(x, dtype=np.float32)
    WQc = WQ.reshape(4, HD, D).sum(0)
    bQc = bQ.reshape(4, HD).sum(0)
    projc = proj.reshape(4, HD, D).sum(0)

    WQT = np.ascontiguousarray(WQc.T).astype(np.float16)    # [D, HD]
    WKT = np.ascontiguousarray(WK.T).astype(np.float16)
    WVT = np.ascontiguousarray(WV.T).astype(np.float16)
    PRJ = np.ascontiguousarray(projc).astype(np.float16)    # [HD, D]
    bQc = np.ascontiguousarray(bQc.reshape(HD, 1), dtype=np.float32)
    bKc = np.ascontiguousarray(bK.reshape(HD, 1), dtype=np.float32)
    bVf = np.ascontiguousarray(
        np.broadcast_to(bV.reshape(1, HD), (128, HD))).astype(np.float16)
    idt = np.eye(128, dtype=np.float16)

    in_maps = []
    for core in range(NCORES):
        b, q = divmod(core, 2)
        xT = np.ascontiguousarray(x[b].T).astype(np.float16)    # [D, T]
        rows = np.concatenate(
            [np.arange(256 * g + 128 * q, 256 * g + 128 * q + 128)
             for g in range(RB)])
        xTq = np.ascontiguousarray(xT[:, rows])     # [D, 1024]
        msk = np.zeros((RB, 128, 512), dtype=np.float32)
        for g in range(RB):
            ntile = KT_TILES[g]
            base = 512 * (ntile - 1)                # keys covered by last tile
            key = base + np.arange(512)[None, :]
            row = (256 * g + 128 * q + np.arange(128))[:, None]
            msk[g] = np.where(key <= row, 0.0, NEG)
        in_maps.append({
            "xT": xT, "xTq": xTq, "WKT": WKT, "WVT": WVT, "WQT": WQT,
            "PRJ": PRJ, "bK": bKc, "bQ": bQc, "BVF": bVf,
            "IDT": idt, "MSK": msk,
        })
    return in_maps


def assemble(results):
    """Gather per-core [1024, D] outputs into [B, T, D]."""
    y = np.empty((B, T, D), dtype=np.float32)
    for core in range(NCORES):
        b, q = divmod(core, 2)
        o = results[core]["out"]
        for g in range(RB):
            y[b, 256 * g + 128 * q:256 * g + 128 * q + 128] = \
                o[128 * g:128 * g + 128]
    return y


_NC_CACHE = None


def kernel(x, WQ, bQ, WK, bK, WV, bV, proj):
    global _NC_CACHE
    in_maps = host_prep(np.asarray(x), np.asarray(WQ), np.asarray(bQ),
                        np.asarray(WK), np.asarray(bK), np.asarray(WV),
                        np.asarray(bV), np.asarray(proj))
    if _NC_CACHE is None:
        _NC_CACHE = build_kernel()
    res = run_bass_kernel_spmd(_NC_CACHE, in_maps, list(range(NCORES)))
    return assemble(res.results)
